# revision 6
# baseline (speedup 1.0000x reference)
"""Multi-head attention (B=2, S=2048, D=1024, H=16) on 8 TRN2 NeuronCores.

Sharding: tensor-parallel over heads x data-parallel over batch.
Core c handles batch b = c // 4 and head group g = c % 4 (4 heads each).
Each core computes its 4 heads' q/k/v projections, attention, and the
partial output projection against its slice of Wo; the host sums the 4
partials per batch element.

Per-core kernel layout:
  - inputs: xT [1024, 2048] (= x[b].T), wq/wk/wv [1024, 256] (= W[rows].T),
    wo [256, 1024] (= Wo[:, cols].T)
  - QT/KT/VT computed transposed ([head-feat, seq]) so the Dh-contraction
    of q@k^T has its contraction dim on partitions.
  - scores are computed transposed ([keys, q]) for a head PAIR; the two
    K=64 score matmuls auto-derive row-tile positions (0,0)/(64,0) and run
    concurrently on the PE; one wide exp via ACT (scale folded); attn @ v
    contracts keys on partitions; columns 64:128 of the v operand hold
    ones so the same matmul emits softmax row-sums replicated across 64
    psum rows.

v2 structure (the scalar engine's exp stream is the critical resource:
128 exps x ~1.1us = ~143us of ACT time; everything else must hide
behind it):
  - projections pipelined at 512-column chunk granularity in k,q,v order
    so the first exp issues ~35us earlier than the v1 phase ordering;
    attention for key-chunk n starts as soon as chunk n of K/V and chunk
    0 of Q are projected.
  - every psum evacuation is an explicit nc.vector copy; nc.any copies
    land on the Scalar engine and stall the exp stream.
  - output projection + store interleaved per q-chunk as soon as both
    head-pairs' normalized outputs exist, instead of a serial tail.
  - fp16 output (host sums the 4 partials in f32): halves store DMA.

fp16 streaming: matmul operands are fp16 (1 cyc/row on the PE vs ~1.6 for
f32r, half the LDWEIGHTS and SBUF cost) while every accumulation stays
f32 in PSUM. Value ranges fit fp16 comfortably (|q|,|k| ~ N(0,1),
exp(scores*scale) <= ~e^7; fp16 max is 65504).
"""

import numpy as np

B, S, D, H, DH = 2, 2048, 1024, 16, 64
NCORES = 8
GROUPS = 4  # head groups; 4 heads = 256 features per core
M = 256  # head features per core
SCALE = 0.125  # 1/sqrt(64)

# stream dtypes per matmul group: "f32r", "bf16", or "fp16"
CFG = {
    "proj": "fp16",   # xT, wq/wk/wv
    "scores": "fp16",  # QT, KT
    "av": "fp16",      # VA, exp tiles
    "wo": "fp16",      # OT, wo
    "out": "fp16",     # output staging + DMA; host sums partials in f32
}

_compiled = None


def _dt(mybir, name):
    return {"f32r": mybir.dt.float32r, "bf16": mybir.dt.bfloat16,
            "fp16": mybir.dt.float16, "f32": mybir.dt.float32}[name]


def _np_dt(name):
    if name == "bf16":
        import ml_dtypes
        return ml_dtypes.bfloat16
    if name == "fp16":
        return np.float16
    return np.float32


def _build_module():
    import concourse.mybir as mybir
    import concourse.tile as tile
    from concourse import bacc

    in_dt = _dt(mybir, CFG["proj"])
    wo_dt = _dt(mybir, CFG["wo"])
    out_dt = _dt(mybir, CFG["out"])
    nc = bacc.Bacc("TRN2", target_bir_lowering=False, debug=False,
                   num_devices=NCORES)
    xT = nc.dram_tensor("xT", [D, S], in_dt, kind="ExternalInput").ap()
    wq = nc.dram_tensor("wq", [D, M], in_dt, kind="ExternalInput").ap()
    wk = nc.dram_tensor("wk", [D, M], in_dt, kind="ExternalInput").ap()
    wv = nc.dram_tensor("wv", [D, M], in_dt, kind="ExternalInput").ap()
    wo = nc.dram_tensor("wo", [M, D], wo_dt, kind="ExternalInput").ap()
    out = nc.dram_tensor("out", [S, D], out_dt, kind="ExternalOutput").ap()

    with tile.TileContext(nc) as tc:
        _kernel_body(tc, out, xT, wq, wk, wv, wo)
    nc.compile()
    return nc


def _kernel_body(tc, out, xT, wq, wk, wv, wo):
    from contextlib import ExitStack

    import concourse.mybir as mybir
    from concourse.masks import make_identity

    nc = tc.nc
    f32 = mybir.dt.float32
    f32r = mybir.dt.float32r
    sc_dt = _dt(mybir, CFG["scores"])
    av_dt = _dt(mybir, CFG["av"])
    wo_dt = _dt(mybir, CFG["wo"])
    AF = mybir.ActivationFunctionType
    AL = mybir.AluOpType

    P = 128
    NKT = D // P   # 8 k-tiles in the projection contraction
    NPT = M // P   # 2 partition-tiles of head features
    SKT = S // P   # 16 key tiles
    QC = 512       # q chunk (psum bank width in f32)
    NQC = S // QC  # 4
    KPC = SKT // NQC  # 4 key tiles per x-chunk
    HPC = 4        # heads per core

    with ExitStack() as ctx:
        const = ctx.enter_context(tc.tile_pool(name="const", bufs=1))
        big = ctx.enter_context(tc.tile_pool(name="big", bufs=1))
        wpool = ctx.enter_context(tc.tile_pool(name="w", bufs=1))
        projin = ctx.enter_context(tc.tile_pool(name="projin", bufs=1))
        work = ctx.enter_context(tc.tile_pool(name="work", bufs=2))
        exp_pool = ctx.enter_context(tc.tile_pool(name="exp", bufs=8))
        small = ctx.enter_context(tc.tile_pool(name="small", bufs=2))
        # PSUM budget (8 banks): psA 2x1 + psS 2x2 + psO 2x1 = 8
        psum_big = ctx.enter_context(tc.tile_pool(name="psA", bufs=2, space="PSUM"))
        psum_s = ctx.enter_context(tc.tile_pool(name="psS", bufs=2, space="PSUM"))
        psum_o = ctx.enter_context(tc.tile_pool(name="psO", bufs=1, space="PSUM"))

        # ---- input DMAs first: weights (gpsimd queue) + xT chunks split
        # across the sync and scalar queues so chunk 0 lands ASAP ----
        w_sb = {}
        for name, w in (("k", wk), ("q", wq), ("v", wv)):
            t = projin.tile([P, NKT, M], w.dtype, tag=f"w{name}")
            nc.gpsimd.dma_start(t[:], w.rearrange("(kt p) m -> p kt m", p=P))
            w_sb[name] = t

        xT_sb = projin.tile([P, NKT, S], xT.dtype, tag="xT")
        xT_r = xT.rearrange("(kt p) s -> p kt s", p=P)
        for c in range(NQC):
            npieces = 4 if c == 0 else 2
            n = NKT // npieces
            for kh in range(npieces):
                eng = nc.sync if kh % 2 == 0 else nc.scalar
                eng.dma_start(
                    xT_sb[:, kh * n:(kh + 1) * n, c * QC:(c + 1) * QC],
                    xT_r[:, kh * n:(kh + 1) * n, c * QC:(c + 1) * QC])

        wo_sb = wpool.tile([P, NPT, D], wo_dt, tag="wo")
        nc.gpsimd.dma_start(wo_sb[:], wo.rearrange("(pt p) n -> p pt n", p=P))

        # ---- constants / warmup ----
        ident_f = const.tile([P, P], f32)
        make_identity(nc, ident_f)
        ident = const.tile([P, P], f32r, tag="ident_r")
        nc.vector.tensor_copy(ident[:], ident_f[:])

        # warm the PE clock (HAM) during the input DMA head so the real
        # projections start at 2.4GHz instead of 1.2
        warm_ps = psum_big.tile([P, P], f32, tag="ps_big")
        for _ in range(28):
            nc.tensor.matmul(warm_ps[:], ident[:], ident[:],
                             start=True, stop=True)

        QT = big.tile([P, NPT, S], sc_dt, tag="QT")
        KT = big.tile([P, NPT, S], sc_dt, tag="KT")
        VT = big.tile([P, NPT, S], f32r, tag="VT")
        OT = big.tile([P, NPT, S], wo_dt, tag="OT")
        VA = big.tile([P, HPC, SKT, P], av_dt, tag="VA")
        # ones block (columns 64:128 of the AV stationary) -> row sums.
        # MEMSET can't set 16-bit values, so build a fp16 ones row once and
        # replicate it on the otherwise-idle gpsimd engine (DVE must stay
        # free for the projection-psum evacuations in the shortened head).
        ones32 = const.tile([P, 64], f32, tag="ones32")
        nc.vector.memset(ones32[:], 1.0)
        ones16 = const.tile([P, 64], av_dt, tag="ones16")
        nc.vector.tensor_copy(ones16[:], ones32[:])
        for h in range(HPC):
            for st in range(SKT):
                nc.gpsimd.tensor_copy(VA[:, h, st, 64:128], ones16[:])

        # ---- building blocks ----
        def proj(name, pt, c):
            """PT[f, s] = sum_d w[d, f] * xT[d, s] for one 512-col chunk."""
            dst = {"q": QT, "k": KT, "v": VT}[name]
            ps = psum_big.tile([P, QC], f32, tag="ps_big")
            for kt in range(NKT):
                nc.tensor.matmul(
                    ps[:],
                    w_sb[name][:, kt, pt * P:(pt + 1) * P],
                    xT_sb[:, kt, c * QC:(c + 1) * QC],
                    start=(kt == 0), stop=(kt == NKT - 1),
                )
            nc.vector.tensor_copy(dst[:, pt, c * QC:(c + 1) * QC], ps[:])

        def vtrans(pt, c):
            """VT chunk back to natural layout in VA (values in cols 0:64)."""
            for st in range(KPC * c, KPC * (c + 1)):
                pst = psum_big.tile([P, P], f32r, tag="ps_big")
                nc.tensor.transpose(pst[:], VT[:, pt, st * P:(st + 1) * P],
                                    ident)
                nc.vector.tensor_copy(VA[:, 2 * pt, st, 0:64], pst[:, 0:64])
                nc.vector.tensor_copy(VA[:, 2 * pt + 1, st, 0:64],
                                      pst[:, 64:128])

        def att_groups(p, c, kts, poA, poB):
            """Score + exp + AV for key tiles `kts` of q-chunk c, pair p."""
            cs = slice(c * QC, (c + 1) * QC)
            for kt in kts:
                ks = slice(kt * P, (kt + 1) * P)
                pss = psum_s.tile([P, 2, QC], f32, tag="ps_s")
                nc.tensor.matmul(pss[:, 0, :], KT[0:64, p, ks],
                                 QT[0:64, p, cs], start=True, stop=True)
                nc.tensor.matmul(pss[:, 1, :], KT[64:128, p, ks],
                                 QT[64:128, p, cs], start=True, stop=True)
                et = exp_pool.tile([P, 2, QC], av_dt, tag="exp")
                nc.scalar.activation(et[:], pss[:], AF.Exp, scale=SCALE)
                nc.tensor.matmul(poA[:], VA[:, 2 * p, kt, :], et[:, 0, :],
                                 start=(kt == 0), stop=(kt == SKT - 1))
                nc.tensor.matmul(poB[:], VA[:, 2 * p + 1, kt, :], et[:, 1, :],
                                 start=(kt == 0), stop=(kt == SKT - 1))

        def norm(p, c, poA, poB):
            """softmax-normalize po into OT (all on DVE)."""
            cs = slice(c * QC, (c + 1) * QC)
            for r0, po in ((0, poA), (64, poB)):
                pc = small.tile([P, QC], f32, tag="po_sb")
                nc.vector.tensor_copy(pc[:], po[:])
                sm = small.tile([64, QC], f32, tag="sums")
                nc.vector.tensor_copy(sm[:], pc[64:128, :])
                rb = small.tile([64, QC], f32, tag="recip")
                nc.vector.reciprocal_approx_fast(rb[:], sm[:])
                nc.vector.tensor_tensor(OT[r0:r0 + 64, p, cs], pc[0:64, :],
                                        rb[:], AL.mult)

        def outproj(c):
            """out[s, n] partial for q rows of chunk c (needs both pairs)."""
            for qt in range(KPC * c, KPC * (c + 1)):
                for nch in range(2):
                    ps = psum_big.tile([P, 512], f32, tag="ps_big")
                    for pt in range(NPT):
                        nc.tensor.matmul(
                            ps[:],
                            OT[:, pt, qt * P:(qt + 1) * P],
                            wo_sb[:, pt, nch * 512:(nch + 1) * 512],
                            start=(pt == 0), stop=(pt == NPT - 1),
                        )
                    ot = work.tile([P, 512], _dt(mybir, CFG["out"]),
                                   tag="outstage")
                    nc.vector.tensor_copy(ot[:], ps[:])
                    nc.sync.dma_start(
                        out[qt * P:(qt + 1) * P, nch * 512:(nch + 1) * 512],
                        ot[:])

        # ---- pair 0: chunk-pipelined projections feeding attention ----
        # wave ck: project k/v chunk ck (and q chunk 0), then attention on
        # q-chunk 0 over the key tiles that just became available.
        poA = psum_o.tile([P, QC], f32, tag="ps_oA")
        poB = psum_o.tile([P, QC], f32, tag="ps_oB")
        for ck in range(NQC):
            proj("k", 0, ck)
            if ck == 0:
                proj("q", 0, 0)
            proj("v", 0, ck)
            vtrans(0, ck)
            att_groups(0, 0, range(KPC * ck, KPC * (ck + 1)), poA, poB)
        norm(0, 0, poA, poB)

        # q-chunks 1..3 of pair 0; pair-1 projections hide inside the
        # ACT-bound attention stream (4 key-tile groups per slice).
        filler = [("k", 1, 0), ("k", 1, 1), ("k", 1, 2), ("k", 1, 3),
                  ("v", 1, 0), ("T", 1, 0), ("v", 1, 1), ("T", 1, 1),
                  ("v", 1, 2), ("T", 1, 2), ("v", 1, 3), ("T", 1, 3),
                  ("q", 1, 0)]
        fi = 0
        for c in range(1, NQC):
            poA = psum_o.tile([P, QC], f32, tag="ps_oA")
            poB = psum_o.tile([P, QC], f32, tag="ps_oB")
            proj("q", 0, c)
            for k4 in range(NQC):
                att_groups(0, c, range(KPC * k4, KPC * (k4 + 1)), poA, poB)
                if fi < len(filler):
                    kind, pt, cc = filler[fi]
                    fi += 1
                    if kind == "T":
                        vtrans(pt, cc)
                    else:
                        proj(kind, pt, cc)
            norm(0, c, poA, poB)
        while fi < len(filler):  # anything not yet emitted
            kind, pt, cc = filler[fi]
            fi += 1
            vtrans(pt, cc) if kind == "T" else proj(kind, pt, cc)

        # ---- pair 1 attention; q-proj, output projection and store of
        # chunk c-1 interleave with chunk c's ACT-bound stream ----
        for c in range(NQC):
            poA = psum_o.tile([P, QC], f32, tag="ps_oA")
            poB = psum_o.tile([P, QC], f32, tag="ps_oB")
            for k4 in range(NQC):
                att_groups(1, c, range(KPC * k4, KPC * (k4 + 1)), poA, poB)
                if k4 == 0 and c + 1 < NQC:
                    proj("q", 1, c + 1)
                elif k4 == 2 and c > 0:
                    outproj(c - 1)
            norm(1, c, poA, poB)
        outproj(NQC - 1)


def _in_maps(x, Wq, Wk, Wv, Wo):
    in_np = _np_dt(CFG["proj"])
    wo_np = _np_dt(CFG["wo"])
    x = np.asarray(x, dtype=np.float32)
    Wq = np.asarray(Wq, dtype=np.float32)
    Wk = np.asarray(Wk, dtype=np.float32)
    Wv = np.asarray(Wv, dtype=np.float32)
    Wo = np.asarray(Wo, dtype=np.float32)
    xT = [np.ascontiguousarray(x[b].T).astype(in_np) for b in range(B)]
    maps = []
    for c in range(NCORES):
        b, g = c // GROUPS, c % GROUPS
        rows = slice(g * M, (g + 1) * M)
        maps.append({
            "xT": xT[b],
            "wq": np.ascontiguousarray(Wq[rows, :].T).astype(in_np),
            "wk": np.ascontiguousarray(Wk[rows, :].T).astype(in_np),
            "wv": np.ascontiguousarray(Wv[rows, :].T).astype(in_np),
            "wo": np.ascontiguousarray(Wo[:, rows].T).astype(wo_np),
        })
    return maps


def kernel(x, Wq, Wk, Wv, Wo, _trace=False):
    global _compiled
    if _compiled is None:
        _compiled = _build_module()
    from concourse.bass_utils import run_bass_kernel_spmd

    res = run_bass_kernel_spmd(
        _compiled, _in_maps(x, Wq, Wk, Wv, Wo),
        core_ids=list(range(NCORES)), trace=_trace,
    )
    outs = [r["out"] for r in res.results]
    y = np.empty((B, S, D), np.float32)
    for b in range(B):
        y[b] = (outs[4 * b].astype(np.float32)
                + outs[4 * b + 1].astype(np.float32)
                + outs[4 * b + 2].astype(np.float32)
                + outs[4 * b + 3].astype(np.float32))
    if _trace:
        kernel.last_results = res
    return y
